# revision 40
# baseline (speedup 1.0000x reference)
"""MSRSA multi-head attention kernel for 8 Trainium2 NeuronCores.

Strategy: data-parallel over batch (B=8 -> 1 batch element per core).
Per core, for its batch element b:
  Qt = (W_q/8) @ queries^T        [512,1024]  (scale 1/8 folded into W_q)
  Kt = W_k @ keys^T               [512,1024]  (stored zero-padded per head)
  V  = values @ W_v^T             [1024,512]  (rows masked by attention_mask)
  per head h, scores are computed TRANSPOSED: S_T[k,q]:
     S_T = sum_d Kt[d,k]*Qt[d,q] + biasT[h][k,q]
  where biasT[h] = (lambda_a[h]*A + lambda_d[h]*D)^T is combined on the host
  (fp16) and streamed from DRAM; it is injected with a single full-rate
  identity matmul per k-tile.  The QK matmul uses 128-row zero-padded Kt
  weights because 64-row weight tiles run the PE at half rate.
  expS = exp(S_T) on ScalarE (PSUM -> SBUF evacuation is the exp)
  attnT_h[d,q] (+ denominator row) = sum_k V_ext[k, d|mask] * expS[k,q]
  (mask column of V_ext -> row 64 of PV output = softmax denominator)
  normalize via reciprocal_approx_fast + K=1 ones-matmul partition bcast
  out = attnT contracted with W_o^T   [1024, 512]

Matmul operands are fp16; accumulation is fp32 in PSUM; exp runs in fp32.
Transposes and the lambda*A+lambda*D combination are host-side marshalling.
"""

import contextlib

import numpy as np

import concourse.bass as bass
import concourse.mybir as mybir
import concourse.tile as tile
from concourse.bass_utils import run_bass_kernel_spmd

B, L, DIN, DM, H = 8, 1024, 256, 512, 8
DH = DM // H  # 64
P = 128
NKT = L // P          # 8 k-tiles
NQC = 2               # q chunks
QC = L // NQC         # 512
F32 = mybir.dt.float32
F16 = mybir.dt.float16


def _emit(tc):
    nc = tc.nc

    def dram(name, shape, dtype=F16, kind="ExternalInput"):
        return nc.dram_tensor(name, shape, dtype, kind=kind).ap()

    qT = dram("qT", [DIN, L])
    kT = dram("kT", [DIN, L])
    vT = dram("vT", [DIN, L])
    wqT = dram("wqT", [DIN, DM])
    wkT = dram("wkT", [DIN, DM])
    wvT = dram("wvT", [DIN, DM])
    woT = dram("woT", [DM, DM])
    biasT = dram("biasT", [P, H * NKT * L])  # [p, h, kt, q] combined bias^T
    identp = dram("identp", [P, P])
    bcastw = dram("bcastw", [P, DH])  # row DH-64... row 64 = ones, else 0
    mask01 = dram("mask01", [P, NKT], F32)
    out = dram("out", [L, DM], F32, kind="ExternalOutput")

    biasT_r = biasT.rearrange("p (h t q) -> p h t q", h=H, t=NKT)

    with contextlib.ExitStack() as ctx:
        singles = ctx.enter_context(tc.tile_pool(name="singles", bufs=1))
        big = ctx.enter_context(tc.tile_pool(name="big", bufs=1))
        bias_pool = ctx.enter_context(tc.tile_pool(name="bias", bufs=4))
        exps = ctx.enter_context(tc.tile_pool(name="exps", bufs=2))
        small = ctx.enter_context(tc.tile_pool(name="small", bufs=3))
        spsum = ctx.enter_context(tc.tile_pool(name="spsum", bufs=3, space="PSUM"))
        pvps = ctx.enter_context(tc.tile_pool(name="pvps", bufs=2, space="PSUM"))

        # ---- small constants (scalar queue: ACT is idle this early) ----
        mask_sb = singles.tile([P, NKT], F32, tag="mask")
        nc.scalar.dma_start(out=mask_sb[:], in_=mask01[:])
        ident_sb = singles.tile([P, P], F16, tag="ident")
        nc.scalar.dma_start(out=ident_sb[:], in_=identp[:])
        bcast_sb = singles.tile([P, DH], F16, tag="bcast")
        nc.scalar.dma_start(out=bcast_sb[:], in_=bcastw[:])

        # ---- big SBUF-resident tensors ----
        qt_sb = big.tile([P, 4, L], F16, tag="qt")   # [p,t,l] = Qt[t*128+p, l]
        # zero-padded per-head Kt: kt_z[:, h, :] has head h's 64 rows at
        # partitions (h%2)*64..+64, zeros elsewhere (full-rate 128-row lhsT)
        kt_z = big.tile([P, H, L], F16, tag="ktz")
        nc.gpsimd.memset(kt_z[:], 0.0)
        vx_sb = big.tile([P, NKT, H, DH + 1], F16, tag="vx")  # V + mask column
        attnT_sb = [
            big.tile([P, 4, QC], F16, tag=f"attnT{qc}", name=f"attnT{qc}")
            for qc in range(NQC)
        ]

        # ---- bias stream: all 16MB on the sync queue (the sync engine
        # feeds its DGE at full rate; the ACT-fed queue starves once exp
        # starts).  q/wq were queued just ahead so the first projection
        # can begin while the bias streams behind it. ----
        bias_tiles = {}

        def fetch_bias(h):
            t = bias_pool.tile([P, NKT, L], F16, tag="bias", name=f"bias{h}")
            if h < 2:
                for kt in range(NKT):
                    nc.sync.dma_start(out=t[:, kt, :], in_=biasT_r[:, h, kt, :])
            else:
                nc.sync.dma_start(out=t[:], in_=biasT_r[:, h])
            bias_tiles[h] = t

        # ---- phase 1: projections (pools scoped so SBUF is reclaimed) ----
        proj_ctx = contextlib.ExitStack()
        stage = proj_ctx.enter_context(tc.tile_pool(name="stage", bufs=3))
        wpool = proj_ctx.enter_context(tc.tile_pool(name="wpool", bufs=3))

        def load_stage(src, eng):
            t = stage.tile([P, 2, L], F16, tag="stage")
            eng.dma_start(out=t[:], in_=src.rearrange("(t p) l -> p t l", p=P))
            return t

        def load_w(src, eng):
            t = wpool.tile([P, 2, DM], F16, tag="w")
            eng.dma_start(out=t[:], in_=src.rearrange("(t p) d -> p t d", p=P))
            return t

        # q/wq lead the sync queue (gate the first matmuls), then the bias
        # stream owns it; everything else rides the scalar queue early.
        q_sb, wq_sb = load_stage(qT, nc.sync), load_w(wqT, nc.sync)
        for h in range(4):
            fetch_bias(h)
        k_sb, wk_sb = load_stage(kT, nc.scalar), load_w(wkT, nc.scalar)
        v_sb, wv_sb = load_stage(vT, nc.scalar), load_w(wvT, nc.scalar)
        wo_sb = singles.tile([P, 4, DM], F16, tag="wo")
        nc.scalar.dma_start(out=wo_sb[:], in_=woT.rearrange("(t p) d -> p t d", p=P))

        # Qt: out[m=dm-tile, n=l-chunk] = sum_din wqT[din, dm] * qT[din, l]
        for mt in range(4):
            for lc in range(NQC):
                ps = pvps.tile([P, QC], F32, tag="pv")
                for kt2 in range(2):
                    nc.tensor.matmul(
                        ps[:],
                        wq_sb[:, kt2, mt * P : (mt + 1) * P],
                        q_sb[:, kt2, lc * QC : (lc + 1) * QC],
                        start=(kt2 == 0),
                        stop=(kt2 == 1),
                    )
                nc.vector.tensor_copy(
                    out=qt_sb[:, mt, lc * QC : (lc + 1) * QC], in_=ps[:]
                )

        # Kt into kt_z halves (head 2mt at partitions 0:64, 2mt+1 at 64:128)
        for mt in range(4):
            for lc in range(NQC):
                ps = pvps.tile([P, QC], F32, tag="pv")
                for kt2 in range(2):
                    nc.tensor.matmul(
                        ps[:],
                        wk_sb[:, kt2, mt * P : (mt + 1) * P],
                        k_sb[:, kt2, lc * QC : (lc + 1) * QC],
                        start=(kt2 == 0),
                        stop=(kt2 == 1),
                    )
                cs = slice(lc * QC, (lc + 1) * QC)
                nc.scalar.copy(out=kt_z[0:DH, 2 * mt, cs], in_=ps[0:DH, :])
                nc.scalar.copy(out=kt_z[DH:P, 2 * mt + 1, cs], in_=ps[DH:P, :])

        # V: out[m=l-tile, n=dm] = sum_din vT[din, l] * wvT[din, dm]; mask rows
        for lt in range(NKT):
            ps = pvps.tile([P, DM], F32, tag="pv")
            for kt2 in range(2):
                nc.tensor.matmul(
                    ps[:],
                    v_sb[:, kt2, lt * P : (lt + 1) * P],
                    wv_sb[:, kt2, :],
                    start=(kt2 == 0),
                    stop=(kt2 == 1),
                )
            nc.vector.tensor_scalar_mul(
                out=vx_sb[:, lt, :, 0:DH],
                in0=ps.rearrange("p (h d) -> p h d", h=H),
                scalar1=mask_sb[:, lt : lt + 1],
            )
            nc.vector.tensor_copy(
                out=vx_sb[:, lt, :, DH : DH + 1],
                in_=mask_sb[:, lt : lt + 1, None].to_broadcast((P, H, 1)),
            )

        proj_ctx.close()

        # ---- phase 2: attention, head-major; full-L score tiles ----
        # dedicated reciprocal-broadcast staging: only row 64 is ever
        # written; rows 0-63/65-127 stay zero so they meet the zero rows
        # of bcast_sb in the full-rate 128-row broadcast matmul
        rec16_bufs = [
            singles.tile([P, QC], F16, tag=f"rec16{qc}", name=f"rec16{qc}")
            for qc in range(NQC)
        ]
        for t in rec16_bufs:
            nc.vector.memset(t[:], 0.0)
        deferred = []

        def wo_proj(qc):
            for lt in range(QC // P):
                ws = pvps.tile([P, DM], F32, tag="pv")
                for kt4 in range(4):
                    nc.tensor.matmul(
                        ws[:],
                        attnT_sb[qc][:, kt4, lt * P : (lt + 1) * P],
                        wo_sb[:, kt4, :],
                        start=(kt4 == 0),
                        stop=(kt4 == 3),
                    )
                ost = small.tile([P, DM], F32, tag="ost")
                nc.vector.tensor_copy(out=ost[:], in_=ws[:])
                nc.sync.dma_start(
                    out=out[qc * QC + lt * P : qc * QC + (lt + 1) * P, :],
                    in_=ost[:],
                )

        def flush_deferred():
            for fn in deferred:
                fn()
            deferred.clear()

        for h in range(H):
            ht = h // 2
            bias_sb = bias_tiles.pop(h)
            ex = exps.tile([P, NKT, L], F16, tag="ex")
            for kt in range(NKT):
                sp = spsum.tile([P, L], F32, tag="sp")
                for qc in range(NQC):
                    qs = slice(qc * QC, (qc + 1) * QC)
                    nc.tensor.matmul(
                        sp[:, qs],
                        kt_z[:, h, kt * P : (kt + 1) * P],
                        qt_sb[:, ht, qs],
                        start=True,
                        stop=False,
                    )
                    nc.tensor.matmul(
                        sp[:, qs],
                        ident_sb[:],
                        bias_sb[:, kt, qs],
                        start=False,
                        stop=True,
                    )
                if kt == 2 and h + 4 <= H - 1:
                    fetch_bias(h + 4)  # keep 4 bias tiles in flight
                if kt == 4:
                    flush_deferred()  # previous head's bps broadcasts
                nc.scalar.activation(
                    out=ex[:, kt, :], in_=sp[:],
                    func=mybir.ActivationFunctionType.Exp,
                )
            for qc in range(NQC):
                qs = slice(qc * QC, (qc + 1) * QC)
                # PV with appended mask column -> row 64 = softmax denominator
                pv = pvps.tile([P, QC], F32, tag="pv")
                for kt in range(NKT):
                    nc.tensor.matmul(
                        pv[0 : DH + 1, :],
                        vx_sb[:, kt, h, :],
                        ex[:, kt, qs],
                        start=(kt == 0),
                        stop=(kt == NKT - 1),
                    )
                # evacuate PV+denominator to SBUF, scaled by 1/16 so the
                # fp16 reciprocal 16/den stays in the fp16 normal range
                pvs = small.tile([DH + 1, QC], F32, tag="pvs")
                nc.scalar.mul(out=pvs[:], in_=pv[0 : DH + 1, :], mul=0.0625)
                # custom-DVE ucode requires base partition 0: compute the
                # reciprocal over all 65 rows; only row 64 (denom) is used
                rec32 = small.tile([DH + 1, QC], F32, tag="rec32")
                nc.vector.reciprocal_approx_fast(out=rec32[:], in_=pvs[:])
                rec16 = rec16_bufs[qc]
                with nc.allow_low_precision(reason="softmax recip fits fp16"):
                    nc.vector.tensor_copy(
                        out=rec16[DH : DH + 1, :], in_=rec32[DH : DH + 1, :]
                    )

                def norm(h=h, ht=ht, qc=qc, pv=pv, pvs=pvs, rec16=rec16):
                    # broadcast 16/den over rows 0..64 of pv (dead after the
                    # pvs evacuation, which rec16 already depends on)
                    bps = pv[0:DH, :]
                    nc.tensor.matmul(
                        bps,
                        bcast_sb[:],
                        rec16[:],
                        start=True,
                        stop=True,
                    )
                    if h % 2 == 0:
                        nc.vector.tensor_mul(
                            out=attnT_sb[qc][0:DH, ht, :], in0=pvs[0:DH, :],
                            in1=bps,
                        )
                    else:
                        tmp = small.tile([DH, QC], F16, tag="odd")
                        nc.vector.tensor_mul(
                            out=tmp[:], in0=pvs[0:DH, :], in1=bps
                        )
                        nc.sync.dma_start(
                            out=attnT_sb[qc][DH:P, ht, :], in_=tmp[:]
                        )

                if h == H - 1:
                    # last head: emit the norm chain and this q-chunk's
                    # output projection immediately so Wo overlaps the
                    # other chunk's PV instead of stalling at the end
                    norm()
                    wo_proj(qc)
                else:
                    deferred.append(norm)
        flush_deferred()


def build_nc():
    from concourse import bacc

    nc = bacc.Bacc("TRN2", target_bir_lowering=False, debug=False)
    with tile.TileContext(nc) as tc:
        _emit(tc)
    nc.compile()
    return nc


_NC = None


def _get_nc():
    global _NC
    if _NC is None:
        _NC = build_nc()
    return _NC


def make_in_maps(queries, keys, values, attention_mask, adjacency_matrix,
                 distance_matrix, W_q, W_k, W_v, W_o, lambda_a, lambda_d, **kw):
    f = np.float32
    h16 = np.float16
    c = np.ascontiguousarray
    wqT = c((W_q.astype(f) * f(0.125)).T).astype(h16)
    wkT = c(W_k.astype(f).T).astype(h16)
    wvT = c(W_v.astype(f).T).astype(h16)
    woT = c(W_o.astype(f).T).astype(h16)
    identp = np.eye(P, dtype=h16)
    bcastw = np.zeros((P, DH), dtype=h16)
    bcastw[DH, :] = 1.0
    la = np.asarray(lambda_a, f)
    ld = np.asarray(lambda_d, f)

    in_maps = []
    for b in range(B):
        At = adjacency_matrix[b].astype(f).T
        Dt = distance_matrix[b].astype(f).T
        # biasT[p, h, kt, q] = (la[h]*A + ld[h]*D)^T chunked into k-tiles
        bias = (la[:, None, None] * At[None] + ld[:, None, None] * Dt[None])
        bias = bias.astype(h16).reshape(H, NKT, P, L).transpose(2, 0, 1, 3)
        in_maps.append({
            "qT": c(queries[b].astype(f).T).astype(h16),
            "kT": c(keys[b].astype(f).T).astype(h16),
            "vT": c(values[b].astype(f).T).astype(h16),
            "wqT": wqT, "wkT": wkT, "wvT": wvT, "woT": woT,
            "biasT": c(bias.reshape(P, H * NKT * L)),
            "identp": identp, "bcastw": bcastw,
            "mask01": c((attention_mask[b] > 0).astype(f).reshape(NKT, P).T),
        })
    return in_maps


def kernel(queries, keys, values, attention_mask, adjacency_matrix,
           distance_matrix, W_q, W_k, W_v, W_o, lambda_a, lambda_d, **kw):
    nc = _get_nc()
    in_maps = make_in_maps(queries, keys, values, attention_mask,
                           adjacency_matrix, distance_matrix,
                           W_q, W_k, W_v, W_o, lambda_a, lambda_d)
    res = run_bass_kernel_spmd(nc, in_maps, list(range(B)), **kw)
    outs = np.stack([res.results[i]["out"] for i in range(B)]).astype(np.float32)
    return outs


# revision 41
# speedup vs baseline: 1.0328x; 1.0328x over previous
"""MSRSA multi-head attention kernel for 8 Trainium2 NeuronCores.

Strategy: data-parallel over batch (B=8 -> 1 batch element per core).
Per core, for its batch element b:
  Qt = (W_q/8) @ queries^T        [512,1024]  (scale 1/8 folded into W_q)
  Kt = W_k @ keys^T               [512,1024]  (stored zero-padded per head)
  V  = values @ W_v^T             [1024,512]  (rows masked by attention_mask)
  per head h, scores are computed TRANSPOSED: S_T[k,q]:
     S_T = sum_d Kt[d,k]*Qt[d,q] + biasT[h][k,q]
  where biasT[h] = (lambda_a[h]*A + lambda_d[h]*D)^T is combined on the host
  (fp16) and streamed from DRAM; it is injected with a single full-rate
  identity matmul per k-tile.  The QK matmul uses 128-row zero-padded Kt
  weights because 64-row weight tiles run the PE at half rate.
  expS = exp(S_T) on ScalarE (PSUM -> SBUF evacuation is the exp)
  attnT_h[d,q] (+ denominator row) = sum_k V_ext[k, d|mask] * expS[k,q]
  (mask column of V_ext -> row 64 of PV output = softmax denominator)
  normalize via reciprocal_approx_fast + K=1 ones-matmul partition bcast
  out = attnT contracted with W_o^T   [1024, 512]

Matmul operands are fp16; accumulation is fp32 in PSUM; exp runs in fp32.
Transposes and the lambda*A+lambda*D combination are host-side marshalling.
"""

import contextlib

import numpy as np

import concourse.bass as bass
import concourse.mybir as mybir
import concourse.tile as tile
from concourse.bass_utils import run_bass_kernel_spmd

B, L, DIN, DM, H = 8, 1024, 256, 512, 8
DH = DM // H  # 64
P = 128
NKT = L // P          # 8 k-tiles
NQC = 2               # q chunks
QC = L // NQC         # 512
F32 = mybir.dt.float32
F16 = mybir.dt.float16


def _emit(tc):
    nc = tc.nc

    def dram(name, shape, dtype=F16, kind="ExternalInput"):
        return nc.dram_tensor(name, shape, dtype, kind=kind).ap()

    qT = dram("qT", [DIN, L])
    kT = dram("kT", [DIN, L])
    vT = dram("vT", [DIN, L])
    wqT = dram("wqT", [DIN, DM])
    wkT = dram("wkT", [DIN, DM])
    wvT = dram("wvT", [DIN, DM])
    woT = dram("woT", [DM, DM])
    biasT = dram("biasT", [P, H * NKT * L])  # [p, h, kt, q] combined bias^T
    identp = dram("identp", [P, P])
    bcastw = dram("bcastw", [P, DH])  # row DH-64... row 64 = ones, else 0
    mask01 = dram("mask01", [P, NKT], F32)
    out = dram("out", [L, DM], F32, kind="ExternalOutput")

    biasT_r = biasT.rearrange("p (h t q) -> p h t q", h=H, t=NKT)

    with contextlib.ExitStack() as ctx:
        singles = ctx.enter_context(tc.tile_pool(name="singles", bufs=1))
        big = ctx.enter_context(tc.tile_pool(name="big", bufs=1))
        bias_pool = ctx.enter_context(tc.tile_pool(name="bias", bufs=4))
        exps = ctx.enter_context(tc.tile_pool(name="exps", bufs=2))
        small = ctx.enter_context(tc.tile_pool(name="small", bufs=3))
        spsum = ctx.enter_context(tc.tile_pool(name="spsum", bufs=3, space="PSUM"))
        pvps = ctx.enter_context(tc.tile_pool(name="pvps", bufs=2, space="PSUM"))

        # ---- small constants (scalar queue: ACT is idle this early) ----
        mask_sb = singles.tile([P, NKT], F32, tag="mask")
        nc.scalar.dma_start(out=mask_sb[:], in_=mask01[:])
        ident_sb = singles.tile([P, P], F16, tag="ident")
        nc.scalar.dma_start(out=ident_sb[:], in_=identp[:])
        bcast_sb = singles.tile([P, DH], F16, tag="bcast")
        nc.scalar.dma_start(out=bcast_sb[:], in_=bcastw[:])

        # ---- big SBUF-resident tensors ----
        qt_sb = big.tile([P, 4, L], F16, tag="qt")   # [p,t,l] = Qt[t*128+p, l]
        # zero-padded per-head Kt: kt_z[:, h, :] has head h's 64 rows at
        # partitions (h%2)*64..+64, zeros elsewhere (full-rate 128-row lhsT)
        kt_z = big.tile([P, H, L], F16, tag="ktz")
        nc.gpsimd.memset(kt_z[:], 0.0)
        vx_sb = big.tile([P, NKT, H, DH + 1], F16, tag="vx")  # V + mask column
        attnT_sb = [
            big.tile([P, 4, QC], F16, tag=f"attnT{qc}", name=f"attnT{qc}")
            for qc in range(NQC)
        ]

        # ---- bias stream: all 16MB on the sync queue (the sync engine
        # feeds its DGE at full rate; the ACT-fed queue starves once exp
        # starts).  q/wq were queued just ahead so the first projection
        # can begin while the bias streams behind it. ----
        bias_tiles = {}

        def fetch_bias(h):
            t = bias_pool.tile([P, NKT, L], F16, tag="bias", name=f"bias{h}")
            if h < 2:
                for kt in range(NKT):
                    nc.sync.dma_start(out=t[:, kt, :], in_=biasT_r[:, h, kt, :])
            else:
                nc.sync.dma_start(out=t[:], in_=biasT_r[:, h])
            bias_tiles[h] = t

        # ---- phase 1: projections (pools scoped so SBUF is reclaimed) ----
        proj_ctx = contextlib.ExitStack()
        stage = proj_ctx.enter_context(tc.tile_pool(name="stage", bufs=3))
        wpool = proj_ctx.enter_context(tc.tile_pool(name="wpool", bufs=3))

        def load_stage(src, eng):
            t = stage.tile([P, 2, L], F16, tag="stage")
            eng.dma_start(out=t[:], in_=src.rearrange("(t p) l -> p t l", p=P))
            return t

        def load_w(src, eng):
            t = wpool.tile([P, 2, DM], F16, tag="w")
            eng.dma_start(out=t[:], in_=src.rearrange("(t p) d -> p t d", p=P))
            return t

        # q/wq lead the sync queue (gate the first matmuls), then the bias
        # stream owns it; everything else rides the scalar queue early.
        q_sb, wq_sb = load_stage(qT, nc.sync), load_w(wqT, nc.sync)
        for h in range(4):
            fetch_bias(h)
        k_sb, wk_sb = load_stage(kT, nc.scalar), load_w(wkT, nc.scalar)
        v_sb, wv_sb = load_stage(vT, nc.scalar), load_w(wvT, nc.scalar)
        wo_sb = singles.tile([P, 4, DM], F16, tag="wo")
        nc.scalar.dma_start(out=wo_sb[:], in_=woT.rearrange("(t p) d -> p t d", p=P))

        # Qt: out[m=dm-tile, n=l-chunk] = sum_din wqT[din, dm] * qT[din, l]
        for mt in range(4):
            for lc in range(NQC):
                ps = pvps.tile([P, QC], F32, tag="pv")
                for kt2 in range(2):
                    nc.tensor.matmul(
                        ps[:],
                        wq_sb[:, kt2, mt * P : (mt + 1) * P],
                        q_sb[:, kt2, lc * QC : (lc + 1) * QC],
                        start=(kt2 == 0),
                        stop=(kt2 == 1),
                    )
                nc.vector.tensor_copy(
                    out=qt_sb[:, mt, lc * QC : (lc + 1) * QC], in_=ps[:]
                )

        # Kt into kt_z halves (head 2mt at partitions 0:64, 2mt+1 at 64:128)
        for mt in range(4):
            for lc in range(NQC):
                ps = pvps.tile([P, QC], F32, tag="pv")
                for kt2 in range(2):
                    nc.tensor.matmul(
                        ps[:],
                        wk_sb[:, kt2, mt * P : (mt + 1) * P],
                        k_sb[:, kt2, lc * QC : (lc + 1) * QC],
                        start=(kt2 == 0),
                        stop=(kt2 == 1),
                    )
                cs = slice(lc * QC, (lc + 1) * QC)
                nc.vector.tensor_copy(
                    out=kt_z[0:DH, 2 * mt, cs], in_=ps[0:DH, :]
                )
                nc.vector.tensor_copy(
                    out=kt_z[DH:P, 2 * mt + 1, cs], in_=ps[DH:P, :]
                )

        # V: out[m=l-tile, n=dm] = sum_din vT[din, l] * wvT[din, dm]; mask rows
        for lt in range(NKT):
            ps = pvps.tile([P, DM], F32, tag="pv")
            for kt2 in range(2):
                nc.tensor.matmul(
                    ps[:],
                    v_sb[:, kt2, lt * P : (lt + 1) * P],
                    wv_sb[:, kt2, :],
                    start=(kt2 == 0),
                    stop=(kt2 == 1),
                )
            nc.vector.tensor_scalar_mul(
                out=vx_sb[:, lt, :, 0:DH],
                in0=ps.rearrange("p (h d) -> p h d", h=H),
                scalar1=mask_sb[:, lt : lt + 1],
            )
            nc.vector.tensor_copy(
                out=vx_sb[:, lt, :, DH : DH + 1],
                in_=mask_sb[:, lt : lt + 1, None].to_broadcast((P, H, 1)),
            )

        proj_ctx.close()

        # ---- phase 2: attention, head-major; full-L score tiles ----
        # dedicated reciprocal-broadcast staging: only row 64 is ever
        # written; rows 0-63/65-127 stay zero so they meet the zero rows
        # of bcast_sb in the full-rate 128-row broadcast matmul
        rec16_bufs = [
            singles.tile([P, QC], F16, tag=f"rec16{qc}", name=f"rec16{qc}")
            for qc in range(NQC)
        ]
        for t in rec16_bufs:
            nc.vector.memset(t[:], 0.0)
        deferred = []

        def wo_proj(qc):
            for lt in range(QC // P):
                ws = pvps.tile([P, DM], F32, tag="pv")
                for kt4 in range(4):
                    nc.tensor.matmul(
                        ws[:],
                        attnT_sb[qc][:, kt4, lt * P : (lt + 1) * P],
                        wo_sb[:, kt4, :],
                        start=(kt4 == 0),
                        stop=(kt4 == 3),
                    )
                ost = small.tile([P, DM], F32, tag="ost")
                nc.vector.tensor_copy(out=ost[:], in_=ws[:])
                nc.sync.dma_start(
                    out=out[qc * QC + lt * P : qc * QC + (lt + 1) * P, :],
                    in_=ost[:],
                )

        def flush_deferred():
            for fn in deferred:
                fn()
            deferred.clear()

        for h in range(H):
            ht = h // 2
            bias_sb = bias_tiles.pop(h)
            ex = exps.tile([P, NKT, L], F16, tag="ex")
            for kt in range(NKT):
                sp = spsum.tile([P, L], F32, tag="sp")
                for qc in range(NQC):
                    qs = slice(qc * QC, (qc + 1) * QC)
                    nc.tensor.matmul(
                        sp[:, qs],
                        kt_z[:, h, kt * P : (kt + 1) * P],
                        qt_sb[:, ht, qs],
                        start=True,
                        stop=False,
                    )
                    nc.tensor.matmul(
                        sp[:, qs],
                        ident_sb[:],
                        bias_sb[:, kt, qs],
                        start=False,
                        stop=True,
                    )
                if kt == 2 and h + 4 <= H - 1:
                    fetch_bias(h + 4)  # keep 4 bias tiles in flight
                if kt == 4:
                    flush_deferred()  # previous head's bps broadcasts
                nc.scalar.activation(
                    out=ex[:, kt, :], in_=sp[:],
                    func=mybir.ActivationFunctionType.Exp,
                )
            for qc in range(NQC):
                qs = slice(qc * QC, (qc + 1) * QC)
                # PV with appended mask column -> row 64 = softmax denominator
                pv = pvps.tile([P, QC], F32, tag="pv")
                for kt in range(NKT):
                    nc.tensor.matmul(
                        pv[0 : DH + 1, :],
                        vx_sb[:, kt, h, :],
                        ex[:, kt, qs],
                        start=(kt == 0),
                        stop=(kt == NKT - 1),
                    )
                # evacuate PV+denominator to SBUF, scaled by 1/16 so the
                # fp16 reciprocal 16/den stays in the fp16 normal range
                pvs = small.tile([DH + 1, QC], F32, tag="pvs")
                nc.scalar.mul(out=pvs[:], in_=pv[0 : DH + 1, :], mul=0.0625)
                # custom-DVE ucode requires base partition 0: compute the
                # reciprocal over all 65 rows; only row 64 (denom) is used
                rec32 = small.tile([DH + 1, QC], F32, tag="rec32")
                nc.vector.reciprocal_approx_fast(out=rec32[:], in_=pvs[:])
                rec16 = rec16_bufs[qc]
                with nc.allow_low_precision(reason="softmax recip fits fp16"):
                    nc.vector.tensor_copy(
                        out=rec16[DH : DH + 1, :], in_=rec32[DH : DH + 1, :]
                    )

                def norm(h=h, ht=ht, qc=qc, pv=pv, pvs=pvs, rec16=rec16):
                    # broadcast 16/den over rows 0..64 of pv (dead after the
                    # pvs evacuation, which rec16 already depends on)
                    bps = pv[0:DH, :]
                    nc.tensor.matmul(
                        bps,
                        bcast_sb[:],
                        rec16[:],
                        start=True,
                        stop=True,
                    )
                    if h % 2 == 0:
                        nc.vector.tensor_mul(
                            out=attnT_sb[qc][0:DH, ht, :], in0=pvs[0:DH, :],
                            in1=bps,
                        )
                    else:
                        tmp = small.tile([DH, QC], F16, tag="odd")
                        nc.vector.tensor_mul(
                            out=tmp[:], in0=pvs[0:DH, :], in1=bps
                        )
                        nc.sync.dma_start(
                            out=attnT_sb[qc][DH:P, ht, :], in_=tmp[:]
                        )

                if h == H - 1:
                    # last head: emit the norm chain and this q-chunk's
                    # output projection immediately so Wo overlaps the
                    # other chunk's PV instead of stalling at the end
                    norm()
                    wo_proj(qc)
                else:
                    deferred.append(norm)
        flush_deferred()


def build_nc():
    from concourse import bacc

    nc = bacc.Bacc("TRN2", target_bir_lowering=False, debug=False)
    with tile.TileContext(nc) as tc:
        _emit(tc)
    nc.compile()
    return nc


_NC = None


def _get_nc():
    global _NC
    if _NC is None:
        _NC = build_nc()
    return _NC


def make_in_maps(queries, keys, values, attention_mask, adjacency_matrix,
                 distance_matrix, W_q, W_k, W_v, W_o, lambda_a, lambda_d, **kw):
    f = np.float32
    h16 = np.float16
    c = np.ascontiguousarray
    wqT = c((W_q.astype(f) * f(0.125)).T).astype(h16)
    wkT = c(W_k.astype(f).T).astype(h16)
    wvT = c(W_v.astype(f).T).astype(h16)
    woT = c(W_o.astype(f).T).astype(h16)
    identp = np.eye(P, dtype=h16)
    bcastw = np.zeros((P, DH), dtype=h16)
    bcastw[DH, :] = 1.0
    la = np.asarray(lambda_a, f)
    ld = np.asarray(lambda_d, f)

    in_maps = []
    for b in range(B):
        At = adjacency_matrix[b].astype(f).T
        Dt = distance_matrix[b].astype(f).T
        # biasT[p, h, kt, q] = (la[h]*A + ld[h]*D)^T chunked into k-tiles
        bias = (la[:, None, None] * At[None] + ld[:, None, None] * Dt[None])
        bias = bias.astype(h16).reshape(H, NKT, P, L).transpose(2, 0, 1, 3)
        in_maps.append({
            "qT": c(queries[b].astype(f).T).astype(h16),
            "kT": c(keys[b].astype(f).T).astype(h16),
            "vT": c(values[b].astype(f).T).astype(h16),
            "wqT": wqT, "wkT": wkT, "wvT": wvT, "woT": woT,
            "biasT": c(bias.reshape(P, H * NKT * L)),
            "identp": identp, "bcastw": bcastw,
            "mask01": c((attention_mask[b] > 0).astype(f).reshape(NKT, P).T),
        })
    return in_maps


def kernel(queries, keys, values, attention_mask, adjacency_matrix,
           distance_matrix, W_q, W_k, W_v, W_o, lambda_a, lambda_d, **kw):
    nc = _get_nc()
    in_maps = make_in_maps(queries, keys, values, attention_mask,
                           adjacency_matrix, distance_matrix,
                           W_q, W_k, W_v, W_o, lambda_a, lambda_d)
    res = run_bass_kernel_spmd(nc, in_maps, list(range(B)), **kw)
    outs = np.stack([res.results[i]["out"] for i in range(B)]).astype(np.float32)
    return outs


# revision 42
# speedup vs baseline: 1.0797x; 1.0453x over previous
"""MSRSA multi-head attention kernel for 8 Trainium2 NeuronCores.

Strategy: data-parallel over batch (B=8 -> 1 batch element per core).
Per core, for its batch element b:
  Qt = (W_q/8) @ queries^T        [512,1024]  (scale 1/8 folded into W_q)
  Kt = W_k @ keys^T               [512,1024]  (stored zero-padded per head)
  V  = values @ W_v^T             [1024,512]  (rows masked by attention_mask)
  per head h, scores are computed TRANSPOSED: S_T[k,q]:
     S_T = sum_d Kt[d,k]*Qt[d,q] + biasT[h][k,q]
  where biasT[h] = (lambda_a[h]*A + lambda_d[h]*D)^T is combined on the host
  (fp16) and streamed from DRAM; it is injected with a single full-rate
  identity matmul per k-tile.  The QK matmul uses 128-row zero-padded Kt
  weights because 64-row weight tiles run the PE at half rate.
  expS = exp(S_T) on ScalarE (PSUM -> SBUF evacuation is the exp)
  attnT_h[d,q] (+ denominator row) = sum_k V_ext[k, d|mask] * expS[k,q]
  (mask column of V_ext -> row 64 of PV output = softmax denominator)
  normalize via reciprocal_approx_fast + K=1 ones-matmul partition bcast
  out = attnT contracted with W_o^T   [1024, 512]

Matmul operands are fp16; accumulation is fp32 in PSUM; exp runs in fp32.
Transposes and the lambda*A+lambda*D combination are host-side marshalling.
"""

import contextlib

import numpy as np

import concourse.bass as bass
import concourse.mybir as mybir
import concourse.tile as tile
from concourse.bass_utils import run_bass_kernel_spmd

B, L, DIN, DM, H = 8, 1024, 256, 512, 8
DH = DM // H  # 64
P = 128
NKT = L // P          # 8 k-tiles
NQC = 2               # q chunks
QC = L // NQC         # 512
F32 = mybir.dt.float32
F16 = mybir.dt.float16


def _emit(tc):
    nc = tc.nc

    def dram(name, shape, dtype=F16, kind="ExternalInput"):
        return nc.dram_tensor(name, shape, dtype, kind=kind).ap()

    qT = dram("qT", [DIN, L])
    kT = dram("kT", [DIN, L])
    vT = dram("vT", [DIN, L])
    wqT = dram("wqT", [DIN, DM])
    wkT = dram("wkT", [DIN, DM])
    wvT = dram("wvT", [DIN, DM])
    woT = dram("woT", [DM, DM])
    biasT = dram("biasT", [P, H * NKT * L])  # [p, h, kt, q] combined bias^T
    identp = dram("identp", [P, P])
    bcastw = dram("bcastw", [P, DH])  # row DH-64... row 64 = ones, else 0
    mask01 = dram("mask01", [P, NKT], F32)
    out = dram("out", [L, DM], F32, kind="ExternalOutput")

    biasT_r = biasT.rearrange("p (h t q) -> p h t q", h=H, t=NKT)

    with contextlib.ExitStack() as ctx:
        singles = ctx.enter_context(tc.tile_pool(name="singles", bufs=1))
        big = ctx.enter_context(tc.tile_pool(name="big", bufs=1))
        bias_pool = ctx.enter_context(tc.tile_pool(name="bias", bufs=4))
        exps = ctx.enter_context(tc.tile_pool(name="exps", bufs=2))
        small = ctx.enter_context(tc.tile_pool(name="small", bufs=3))
        spsum = ctx.enter_context(tc.tile_pool(name="spsum", bufs=3, space="PSUM"))
        pvps = ctx.enter_context(tc.tile_pool(name="pvps", bufs=2, space="PSUM"))

        # ---- small constants (scalar queue: ACT is idle this early) ----
        mask_sb = singles.tile([P, NKT], F32, tag="mask")
        nc.scalar.dma_start(out=mask_sb[:], in_=mask01[:])
        ident_sb = singles.tile([P, P], F16, tag="ident")
        nc.scalar.dma_start(out=ident_sb[:], in_=identp[:])
        bcast_sb = singles.tile([P, DH], F16, tag="bcast")
        nc.scalar.dma_start(out=bcast_sb[:], in_=bcastw[:])

        # ---- big SBUF-resident tensors ----
        qt_sb = big.tile([P, 4, L], F16, tag="qt")   # [p,t,l] = Qt[t*128+p, l]
        # zero-padded per-head Kt: kt_z[:, h, :] has head h's 64 rows at
        # partitions (h%2)*64..+64, zeros elsewhere (full-rate 128-row lhsT)
        kt_z = big.tile([P, H, L], F16, tag="ktz")
        nc.gpsimd.memset(kt_z[:], 0.0)
        vx_sb = big.tile([P, NKT, H, DH + 1], F16, tag="vx")  # V + mask column
        attnT_sb = [
            big.tile([P, 4, QC], F16, tag=f"attnT{qc}", name=f"attnT{qc}")
            for qc in range(NQC)
        ]

        # ---- bias stream: all 16MB on the sync queue (the sync engine
        # feeds its DGE at full rate; the ACT-fed queue starves once exp
        # starts).  q/wq were queued just ahead so the first projection
        # can begin while the bias streams behind it. ----
        bias_tiles = {}

        def fetch_bias(h):
            t = bias_pool.tile([P, NKT, L], F16, tag="bias", name=f"bias{h}")
            if h < 2:
                for kt in range(NKT):
                    nc.sync.dma_start(out=t[:, kt, :], in_=biasT_r[:, h, kt, :])
            else:
                nc.sync.dma_start(out=t[:], in_=biasT_r[:, h])
            bias_tiles[h] = t

        # ---- phase 1: projections (pools scoped so SBUF is reclaimed) ----
        proj_ctx = contextlib.ExitStack()
        stage = proj_ctx.enter_context(tc.tile_pool(name="stage", bufs=3))
        wpool = proj_ctx.enter_context(tc.tile_pool(name="wpool", bufs=3))

        def load_stage(src, eng):
            t = stage.tile([P, 2, L], F16, tag="stage")
            eng.dma_start(out=t[:], in_=src.rearrange("(t p) l -> p t l", p=P))
            return t

        def load_w(src, eng):
            t = wpool.tile([P, 2, DM], F16, tag="w")
            eng.dma_start(out=t[:], in_=src.rearrange("(t p) d -> p t d", p=P))
            return t

        # q/wq lead the sync queue (gate the first matmuls), then the bias
        # stream owns it; everything else rides the scalar queue early.
        q_sb, wq_sb = load_stage(qT, nc.sync), load_w(wqT, nc.sync)
        for h in range(4):
            fetch_bias(h)
        k_sb, wk_sb = load_stage(kT, nc.scalar), load_w(wkT, nc.scalar)
        v_sb, wv_sb = load_stage(vT, nc.scalar), load_w(wvT, nc.scalar)
        wo_sb = singles.tile([P, 4, DM], F16, tag="wo")
        nc.scalar.dma_start(out=wo_sb[:], in_=woT.rearrange("(t p) d -> p t d", p=P))

        # Qt: out[m=dm-tile, n=l-chunk] = sum_din wqT[din, dm] * qT[din, l]
        for mt in range(4):
            for lc in range(NQC):
                ps = pvps.tile([P, QC], F32, tag="pv")
                for kt2 in range(2):
                    nc.tensor.matmul(
                        ps[:],
                        wq_sb[:, kt2, mt * P : (mt + 1) * P],
                        q_sb[:, kt2, lc * QC : (lc + 1) * QC],
                        start=(kt2 == 0),
                        stop=(kt2 == 1),
                    )
                nc.vector.tensor_copy(
                    out=qt_sb[:, mt, lc * QC : (lc + 1) * QC], in_=ps[:]
                )

        # Kt into kt_z halves (head 2mt at partitions 0:64, 2mt+1 at 64:128)
        for mt in range(4):
            for lc in range(NQC):
                ps = pvps.tile([P, QC], F32, tag="pv")
                for kt2 in range(2):
                    nc.tensor.matmul(
                        ps[:],
                        wk_sb[:, kt2, mt * P : (mt + 1) * P],
                        k_sb[:, kt2, lc * QC : (lc + 1) * QC],
                        start=(kt2 == 0),
                        stop=(kt2 == 1),
                    )
                cs = slice(lc * QC, (lc + 1) * QC)
                nc.vector.tensor_copy(
                    out=kt_z[0:DH, 2 * mt, cs], in_=ps[0:DH, :]
                )
                nc.vector.tensor_copy(
                    out=kt_z[DH:P, 2 * mt + 1, cs], in_=ps[DH:P, :]
                )

        # V: out[m=l-tile, n=dm] = sum_din vT[din, l] * wvT[din, dm]; mask rows
        for lt in range(NKT):
            ps = pvps.tile([P, DM], F32, tag="pv")
            for kt2 in range(2):
                nc.tensor.matmul(
                    ps[:],
                    v_sb[:, kt2, lt * P : (lt + 1) * P],
                    wv_sb[:, kt2, :],
                    start=(kt2 == 0),
                    stop=(kt2 == 1),
                )
            nc.vector.tensor_scalar_mul(
                out=vx_sb[:, lt, :, 0:DH],
                in0=ps.rearrange("p (h d) -> p h d", h=H),
                scalar1=mask_sb[:, lt : lt + 1],
            )
            nc.vector.tensor_copy(
                out=vx_sb[:, lt, :, DH : DH + 1],
                in_=mask_sb[:, lt : lt + 1, None].to_broadcast((P, H, 1)),
            )

        proj_ctx.close()

        # ---- phase 2: attention, head-major; full-L score tiles ----
        # dedicated reciprocal-broadcast staging: only row 64 is ever
        # written; rows 0-63/65-127 stay zero so they meet the zero rows
        # of bcast_sb in the full-rate 128-row broadcast matmul
        rec16_bufs = [
            singles.tile([P, QC], F16, tag=f"rec16{qc}", name=f"rec16{qc}")
            for qc in range(NQC)
        ]
        for t in rec16_bufs:
            nc.vector.memset(t[:], 0.0)
        deferred = []

        def wo_proj(qc):
            for lt in range(QC // P):
                ws = pvps.tile([P, DM], F32, tag="pv")
                for kt4 in range(4):
                    nc.tensor.matmul(
                        ws[:],
                        attnT_sb[qc][:, kt4, lt * P : (lt + 1) * P],
                        wo_sb[:, kt4, :],
                        start=(kt4 == 0),
                        stop=(kt4 == 3),
                    )
                ost = small.tile([P, DM], F32, tag="ost")
                nc.vector.tensor_copy(out=ost[:], in_=ws[:])
                nc.sync.dma_start(
                    out=out[qc * QC + lt * P : qc * QC + (lt + 1) * P, :],
                    in_=ost[:],
                )

        def flush_deferred():
            for fn in deferred:
                fn()
            deferred.clear()

        for h in range(H):
            ht = h // 2
            bias_sb = bias_tiles.pop(h)
            ex = exps.tile([P, NKT, L], F16, tag="ex")
            for kt in range(NKT):
                sp = spsum.tile([P, L], F32, tag="sp")
                for qc in range(NQC):
                    qs = slice(qc * QC, (qc + 1) * QC)
                    nc.tensor.matmul(
                        sp[:, qs],
                        kt_z[:, h, kt * P : (kt + 1) * P],
                        qt_sb[:, ht, qs],
                        start=True,
                        stop=False,
                    )
                    nc.tensor.matmul(
                        sp[:, qs],
                        ident_sb[:],
                        bias_sb[:, kt, qs],
                        start=False,
                        stop=True,
                    )
                if kt == 2 and h + 4 <= H - 1:
                    fetch_bias(h + 4)  # keep 4 bias tiles in flight
                if kt == 4:
                    flush_deferred()  # previous head's bps broadcasts
                nc.scalar.activation(
                    out=ex[:, kt, :], in_=sp[:],
                    func=mybir.ActivationFunctionType.Exp,
                )
            for qc in range(NQC):
                qs = slice(qc * QC, (qc + 1) * QC)
                # PV with appended mask column -> row 64 = softmax denominator
                pv = pvps.tile([P, QC], F32, tag="pv")
                for kt in range(NKT):
                    nc.tensor.matmul(
                        pv[0 : DH + 1, :],
                        vx_sb[:, kt, h, :],
                        ex[:, kt, qs],
                        start=(kt == 0),
                        stop=(kt == NKT - 1),
                    )
                # evacuate PV+denominator to SBUF, scaled by 1/16 so the
                # fp16 reciprocal 16/den stays in the fp16 normal range
                pvs = small.tile([DH + 1, QC], F32, tag="pvs")
                nc.scalar.mul(out=pvs[:], in_=pv[0 : DH + 1, :], mul=0.0625)
                # custom-DVE ucode requires base partition 0: compute the
                # reciprocal over all 65 rows; only row 64 (denom) is used
                rec32 = small.tile([DH + 1, QC], F32, tag="rec32")
                nc.vector.reciprocal_approx_fast(out=rec32[:], in_=pvs[:])
                rec16 = rec16_bufs[qc]
                with nc.allow_low_precision(reason="softmax recip fits fp16"):
                    nc.vector.tensor_copy(
                        out=rec16[DH : DH + 1, :], in_=rec32[DH : DH + 1, :]
                    )

                def norm(h=h, ht=ht, qc=qc, pv=pv, pvs=pvs, rec16=rec16):
                    # broadcast 16/den over rows 0..64 of pv (dead after the
                    # pvs evacuation, which rec16 already depends on)
                    bps = pv[0:DH, :]
                    nc.tensor.matmul(
                        bps,
                        bcast_sb[:],
                        rec16[:],
                        start=True,
                        stop=True,
                    )
                    if h % 2 == 0:
                        nc.vector.tensor_mul(
                            out=attnT_sb[qc][0:DH, ht, :], in0=pvs[0:DH, :],
                            in1=bps,
                        )
                    else:
                        tmp = small.tile([DH, QC], F16, tag="odd")
                        nc.vector.tensor_mul(
                            out=tmp[:], in0=pvs[0:DH, :], in1=bps
                        )
                        nc.sync.dma_start(
                            out=attnT_sb[qc][DH:P, ht, :], in_=tmp[:]
                        )

                if h == H - 1:
                    norm()  # no later PE work can hide it; emit now
                else:
                    deferred.append(norm)
        flush_deferred()
        for qc in range(NQC):
            wo_proj(qc)


def build_nc():
    from concourse import bacc

    nc = bacc.Bacc("TRN2", target_bir_lowering=False, debug=False)
    with tile.TileContext(nc) as tc:
        _emit(tc)
    nc.compile()
    return nc


_NC = None


def _get_nc():
    global _NC
    if _NC is None:
        _NC = build_nc()
    return _NC


def make_in_maps(queries, keys, values, attention_mask, adjacency_matrix,
                 distance_matrix, W_q, W_k, W_v, W_o, lambda_a, lambda_d, **kw):
    f = np.float32
    h16 = np.float16
    c = np.ascontiguousarray
    wqT = c((W_q.astype(f) * f(0.125)).T).astype(h16)
    wkT = c(W_k.astype(f).T).astype(h16)
    wvT = c(W_v.astype(f).T).astype(h16)
    woT = c(W_o.astype(f).T).astype(h16)
    identp = np.eye(P, dtype=h16)
    bcastw = np.zeros((P, DH), dtype=h16)
    bcastw[DH, :] = 1.0
    la = np.asarray(lambda_a, f)
    ld = np.asarray(lambda_d, f)

    in_maps = []
    for b in range(B):
        At = adjacency_matrix[b].astype(f).T
        Dt = distance_matrix[b].astype(f).T
        # biasT[p, h, kt, q] = (la[h]*A + ld[h]*D)^T chunked into k-tiles
        bias = (la[:, None, None] * At[None] + ld[:, None, None] * Dt[None])
        bias = bias.astype(h16).reshape(H, NKT, P, L).transpose(2, 0, 1, 3)
        in_maps.append({
            "qT": c(queries[b].astype(f).T).astype(h16),
            "kT": c(keys[b].astype(f).T).astype(h16),
            "vT": c(values[b].astype(f).T).astype(h16),
            "wqT": wqT, "wkT": wkT, "wvT": wvT, "woT": woT,
            "biasT": c(bias.reshape(P, H * NKT * L)),
            "identp": identp, "bcastw": bcastw,
            "mask01": c((attention_mask[b] > 0).astype(f).reshape(NKT, P).T),
        })
    return in_maps


def kernel(queries, keys, values, attention_mask, adjacency_matrix,
           distance_matrix, W_q, W_k, W_v, W_o, lambda_a, lambda_d, **kw):
    nc = _get_nc()
    in_maps = make_in_maps(queries, keys, values, attention_mask,
                           adjacency_matrix, distance_matrix,
                           W_q, W_k, W_v, W_o, lambda_a, lambda_d)
    res = run_bass_kernel_spmd(nc, in_maps, list(range(B)), **kw)
    outs = np.stack([res.results[i]["out"] for i in range(B)]).astype(np.float32)
    return outs


# revision 43
# speedup vs baseline: 1.0898x; 1.0094x over previous
"""MSRSA multi-head attention kernel for 8 Trainium2 NeuronCores.

Strategy: data-parallel over batch (B=8 -> 1 batch element per core).
Per core, for its batch element b:
  Qt = (W_q/8) @ queries^T        [512,1024]  (scale 1/8 folded into W_q)
  Kt = W_k @ keys^T               [512,1024]  (stored zero-padded per head)
  V  = values @ W_v^T             [1024,512]  (rows masked by attention_mask)
  per head h, scores are computed TRANSPOSED: S_T[k,q]:
     S_T = sum_d Kt[d,k]*Qt[d,q] + biasT[h][k,q]
  where biasT[h] = (lambda_a[h]*A + lambda_d[h]*D)^T is combined on the host
  (fp16) and streamed from DRAM; it is injected with a single full-rate
  identity matmul per k-tile.  The QK matmul uses 128-row zero-padded Kt
  weights because 64-row weight tiles run the PE at half rate.
  expS = exp(S_T) on ScalarE (PSUM -> SBUF evacuation is the exp)
  attnT_h[d,q] (+ denominator row) = sum_k V_ext[k, d|mask] * expS[k,q]
  (mask column of V_ext -> row 64 of PV output = softmax denominator)
  normalize via reciprocal_approx_fast + K=1 ones-matmul partition bcast
  out = attnT contracted with W_o^T   [1024, 512]

Matmul operands are fp16; accumulation is fp32 in PSUM; exp runs in fp32.
Transposes and the lambda*A+lambda*D combination are host-side marshalling.
"""

import contextlib

import numpy as np

import concourse.bass as bass
import concourse.mybir as mybir
import concourse.tile as tile
from concourse.bass_utils import run_bass_kernel_spmd

B, L, DIN, DM, H = 8, 1024, 256, 512, 8
DH = DM // H  # 64
P = 128
NKT = L // P          # 8 k-tiles
NQC = 2               # q chunks
QC = L // NQC         # 512
F32 = mybir.dt.float32
F16 = mybir.dt.float16


def _emit(tc):
    nc = tc.nc

    def dram(name, shape, dtype=F16, kind="ExternalInput"):
        return nc.dram_tensor(name, shape, dtype, kind=kind).ap()

    qT = dram("qT", [DIN, L])
    kT = dram("kT", [DIN, L])
    vT = dram("vT", [DIN, L])
    wqT = dram("wqT", [DIN, DM])
    wkT = dram("wkT", [DIN, DM])
    wvT = dram("wvT", [DIN, DM])
    woT = dram("woT", [DM, DM])
    biasT = dram("biasT", [P, H * NKT * L])  # [p, h, kt, q] combined bias^T
    identp = dram("identp", [P, P])
    bcastw = dram("bcastw", [P, DH])  # row DH-64... row 64 = ones, else 0
    mask01 = dram("mask01", [P, NKT], F32)
    out = dram("out", [L, DM], F32, kind="ExternalOutput")

    biasT_r = biasT.rearrange("p (h t q) -> p h t q", h=H, t=NKT)

    with contextlib.ExitStack() as ctx:
        singles = ctx.enter_context(tc.tile_pool(name="singles", bufs=1))
        big = ctx.enter_context(tc.tile_pool(name="big", bufs=1))
        bias_pool = ctx.enter_context(tc.tile_pool(name="bias", bufs=4))
        exps = ctx.enter_context(tc.tile_pool(name="exps", bufs=2))
        small = ctx.enter_context(tc.tile_pool(name="small", bufs=3))
        spsum = ctx.enter_context(tc.tile_pool(name="spsum", bufs=3, space="PSUM"))
        pvps = ctx.enter_context(tc.tile_pool(name="pvps", bufs=2, space="PSUM"))

        # ---- small constants (scalar queue: ACT is idle this early) ----
        mask_sb = singles.tile([P, NKT], F32, tag="mask")
        nc.scalar.dma_start(out=mask_sb[:], in_=mask01[:])
        ident_sb = singles.tile([P, P], F16, tag="ident")
        nc.scalar.dma_start(out=ident_sb[:], in_=identp[:])
        bcast_sb = singles.tile([P, DH], F16, tag="bcast")
        nc.scalar.dma_start(out=bcast_sb[:], in_=bcastw[:])
        # warm the exp activation table while DMA queues spin up
        warm = singles.tile([1, 8], F32, tag="warm")
        nc.vector.memset(warm[:], 0.0)
        nc.scalar.activation(
            out=warm[:], in_=warm[:], func=mybir.ActivationFunctionType.Exp
        )

        # ---- big SBUF-resident tensors ----
        qt_sb = big.tile([P, 4, L], F16, tag="qt")   # [p,t,l] = Qt[t*128+p, l]
        # zero-padded per-head Kt: kt_z[:, h, :] has head h's 64 rows at
        # partitions (h%2)*64..+64, zeros elsewhere (full-rate 128-row lhsT)
        kt_z = big.tile([P, H, L], F16, tag="ktz")
        nc.gpsimd.memset(kt_z[:], 0.0)
        vx_sb = big.tile([P, NKT, H, DH + 1], F16, tag="vx")  # V + mask column
        attnT_sb = [
            big.tile([P, 4, QC], F16, tag=f"attnT{qc}", name=f"attnT{qc}")
            for qc in range(NQC)
        ]

        # ---- bias stream: all 16MB on the sync queue (the sync engine
        # feeds its DGE at full rate; the ACT-fed queue starves once exp
        # starts).  q/wq were queued just ahead so the first projection
        # can begin while the bias streams behind it. ----
        bias_tiles = {}

        def fetch_bias(h):
            t = bias_pool.tile([P, NKT, L], F16, tag="bias", name=f"bias{h}")
            if h < 2:
                for kt in range(NKT):
                    nc.sync.dma_start(out=t[:, kt, :], in_=biasT_r[:, h, kt, :])
            else:
                nc.sync.dma_start(out=t[:], in_=biasT_r[:, h])
            bias_tiles[h] = t

        # ---- phase 1: projections (pools scoped so SBUF is reclaimed) ----
        proj_ctx = contextlib.ExitStack()
        stage = proj_ctx.enter_context(tc.tile_pool(name="stage", bufs=3))
        wpool = proj_ctx.enter_context(tc.tile_pool(name="wpool", bufs=3))

        def load_stage(src, eng, split=False):
            t = stage.tile([P, 2, L], F16, tag="stage")
            r = src.rearrange("(t p) l -> p t l", p=P)
            if split:
                for j in range(2):
                    eng.dma_start(out=t[:, j], in_=r[:, j])
            else:
                eng.dma_start(out=t[:], in_=r)
            return t

        def load_w(src, eng, split=False):
            t = wpool.tile([P, 2, DM], F16, tag="w")
            r = src.rearrange("(t p) d -> p t d", p=P)
            if split:
                for j in range(2):
                    eng.dma_start(out=t[:, j], in_=r[:, j])
            else:
                eng.dma_start(out=t[:], in_=r)
            return t

        # q/wq lead the sync queue (gate the first matmuls), then the bias
        # stream owns it; everything else rides the scalar queue early.
        # Interleave: wq half 0, q half 0 (first matmul gate), then halves 1.
        wq_sb = wpool.tile([P, 2, DM], F16, tag="w")
        q_sb = stage.tile([P, 2, L], F16, tag="stage")
        wq_r = wqT.rearrange("(t p) d -> p t d", p=P)
        q_r = qT.rearrange("(t p) l -> p t l", p=P)
        for j in range(2):
            nc.sync.dma_start(out=wq_sb[:, j], in_=wq_r[:, j])
            nc.sync.dma_start(out=q_sb[:, j], in_=q_r[:, j])
        for h in range(4):
            fetch_bias(h)
        k_sb, wk_sb = load_stage(kT, nc.scalar), load_w(wkT, nc.scalar)
        v_sb, wv_sb = load_stage(vT, nc.scalar), load_w(wvT, nc.scalar)
        wo_sb = singles.tile([P, 4, DM], F16, tag="wo")
        nc.scalar.dma_start(out=wo_sb[:], in_=woT.rearrange("(t p) d -> p t d", p=P))

        # Qt: out[m=dm-tile, n=l-chunk] = sum_din wqT[din, dm] * qT[din, l]
        for mt in range(4):
            for lc in range(NQC):
                ps = pvps.tile([P, QC], F32, tag="pv")
                for kt2 in range(2):
                    nc.tensor.matmul(
                        ps[:],
                        wq_sb[:, kt2, mt * P : (mt + 1) * P],
                        q_sb[:, kt2, lc * QC : (lc + 1) * QC],
                        start=(kt2 == 0),
                        stop=(kt2 == 1),
                    )
                nc.vector.tensor_copy(
                    out=qt_sb[:, mt, lc * QC : (lc + 1) * QC], in_=ps[:]
                )

        # Kt into kt_z halves (head 2mt at partitions 0:64, 2mt+1 at 64:128)
        for mt in range(4):
            for lc in range(NQC):
                ps = pvps.tile([P, QC], F32, tag="pv")
                for kt2 in range(2):
                    nc.tensor.matmul(
                        ps[:],
                        wk_sb[:, kt2, mt * P : (mt + 1) * P],
                        k_sb[:, kt2, lc * QC : (lc + 1) * QC],
                        start=(kt2 == 0),
                        stop=(kt2 == 1),
                    )
                cs = slice(lc * QC, (lc + 1) * QC)
                nc.vector.tensor_copy(
                    out=kt_z[0:DH, 2 * mt, cs], in_=ps[0:DH, :]
                )
                nc.vector.tensor_copy(
                    out=kt_z[DH:P, 2 * mt + 1, cs], in_=ps[DH:P, :]
                )

        # V: out[m=l-tile, n=dm] = sum_din vT[din, l] * wvT[din, dm]; mask rows
        for lt in range(NKT):
            ps = pvps.tile([P, DM], F32, tag="pv")
            for kt2 in range(2):
                nc.tensor.matmul(
                    ps[:],
                    v_sb[:, kt2, lt * P : (lt + 1) * P],
                    wv_sb[:, kt2, :],
                    start=(kt2 == 0),
                    stop=(kt2 == 1),
                )
            nc.vector.tensor_scalar_mul(
                out=vx_sb[:, lt, :, 0:DH],
                in0=ps.rearrange("p (h d) -> p h d", h=H),
                scalar1=mask_sb[:, lt : lt + 1],
            )
            nc.vector.tensor_copy(
                out=vx_sb[:, lt, :, DH : DH + 1],
                in_=mask_sb[:, lt : lt + 1, None].to_broadcast((P, H, 1)),
            )

        proj_ctx.close()

        # ---- phase 2: attention, head-major; full-L score tiles ----
        # dedicated reciprocal-broadcast staging: only row 64 is ever
        # written; rows 0-63/65-127 stay zero so they meet the zero rows
        # of bcast_sb in the full-rate 128-row broadcast matmul
        rec16_bufs = [
            singles.tile([P, QC], F16, tag=f"rec16{qc}", name=f"rec16{qc}")
            for qc in range(NQC)
        ]
        for t in rec16_bufs:
            nc.vector.memset(t[:], 0.0)
        deferred = []

        def wo_proj(qc):
            for lt in range(QC // P):
                ws = pvps.tile([P, DM], F32, tag="pv")
                for kt4 in range(4):
                    nc.tensor.matmul(
                        ws[:],
                        attnT_sb[qc][:, kt4, lt * P : (lt + 1) * P],
                        wo_sb[:, kt4, :],
                        start=(kt4 == 0),
                        stop=(kt4 == 3),
                    )
                ost = small.tile([P, DM], F32, tag="ost")
                nc.vector.tensor_copy(out=ost[:], in_=ws[:])
                nc.sync.dma_start(
                    out=out[qc * QC + lt * P : qc * QC + (lt + 1) * P, :],
                    in_=ost[:],
                )

        def flush_deferred():
            for fn in deferred:
                fn()
            deferred.clear()

        for h in range(H):
            ht = h // 2
            bias_sb = bias_tiles.pop(h)
            ex = exps.tile([P, NKT, L], F16, tag="ex")
            for kt in range(NKT):
                sp = spsum.tile([P, L], F32, tag="sp")
                for qc in range(NQC):
                    qs = slice(qc * QC, (qc + 1) * QC)
                    nc.tensor.matmul(
                        sp[:, qs],
                        kt_z[:, h, kt * P : (kt + 1) * P],
                        qt_sb[:, ht, qs],
                        start=True,
                        stop=False,
                    )
                    nc.tensor.matmul(
                        sp[:, qs],
                        ident_sb[:],
                        bias_sb[:, kt, qs],
                        start=False,
                        stop=True,
                    )
                if kt == 2 and h + 4 <= H - 1:
                    fetch_bias(h + 4)  # keep 4 bias tiles in flight
                if kt == 4:
                    flush_deferred()  # previous head's bps broadcasts
                nc.scalar.activation(
                    out=ex[:, kt, :], in_=sp[:],
                    func=mybir.ActivationFunctionType.Exp,
                )
            for qc in range(NQC):
                qs = slice(qc * QC, (qc + 1) * QC)
                # PV with appended mask column -> row 64 = softmax denominator
                pv = pvps.tile([P, QC], F32, tag="pv")
                for kt in range(NKT):
                    nc.tensor.matmul(
                        pv[0 : DH + 1, :],
                        vx_sb[:, kt, h, :],
                        ex[:, kt, qs],
                        start=(kt == 0),
                        stop=(kt == NKT - 1),
                    )
                # evacuate PV+denominator to SBUF, scaled by 1/16 so the
                # fp16 reciprocal 16/den stays in the fp16 normal range
                pvs = small.tile([DH + 1, QC], F32, tag="pvs")
                nc.scalar.mul(out=pvs[:], in_=pv[0 : DH + 1, :], mul=0.0625)
                # custom-DVE ucode requires base partition 0: compute the
                # reciprocal over all 65 rows; only row 64 (denom) is used
                rec32 = small.tile([DH + 1, QC], F32, tag="rec32")
                nc.vector.reciprocal_approx_fast(out=rec32[:], in_=pvs[:])
                rec16 = rec16_bufs[qc]
                with nc.allow_low_precision(reason="softmax recip fits fp16"):
                    nc.vector.tensor_copy(
                        out=rec16[DH : DH + 1, :], in_=rec32[DH : DH + 1, :]
                    )

                def norm(h=h, ht=ht, qc=qc, pv=pv, pvs=pvs, rec16=rec16):
                    # broadcast 16/den over rows 0..64 of pv (dead after the
                    # pvs evacuation, which rec16 already depends on)
                    bps = pv[0:DH, :]
                    nc.tensor.matmul(
                        bps,
                        bcast_sb[:],
                        rec16[:],
                        start=True,
                        stop=True,
                    )
                    if h % 2 == 0:
                        nc.vector.tensor_mul(
                            out=attnT_sb[qc][0:DH, ht, :], in0=pvs[0:DH, :],
                            in1=bps,
                        )
                    else:
                        tmp = small.tile([DH, QC], F16, tag="odd")
                        nc.vector.tensor_mul(
                            out=tmp[:], in0=pvs[0:DH, :], in1=bps
                        )
                        nc.sync.dma_start(
                            out=attnT_sb[qc][DH:P, ht, :], in_=tmp[:]
                        )

                if h == H - 1:
                    norm()  # no later PE work can hide it; emit now
                else:
                    deferred.append(norm)
        flush_deferred()
        for qc in range(NQC):
            wo_proj(qc)


def build_nc():
    from concourse import bacc

    nc = bacc.Bacc("TRN2", target_bir_lowering=False, debug=False)
    with tile.TileContext(nc) as tc:
        _emit(tc)
    nc.compile()
    return nc


_NC = None


def _get_nc():
    global _NC
    if _NC is None:
        _NC = build_nc()
    return _NC


def make_in_maps(queries, keys, values, attention_mask, adjacency_matrix,
                 distance_matrix, W_q, W_k, W_v, W_o, lambda_a, lambda_d, **kw):
    f = np.float32
    h16 = np.float16
    c = np.ascontiguousarray
    wqT = c((W_q.astype(f) * f(0.125)).T).astype(h16)
    wkT = c(W_k.astype(f).T).astype(h16)
    wvT = c(W_v.astype(f).T).astype(h16)
    woT = c(W_o.astype(f).T).astype(h16)
    identp = np.eye(P, dtype=h16)
    bcastw = np.zeros((P, DH), dtype=h16)
    bcastw[DH, :] = 1.0
    la = np.asarray(lambda_a, f)
    ld = np.asarray(lambda_d, f)

    in_maps = []
    for b in range(B):
        At = adjacency_matrix[b].astype(f).T
        Dt = distance_matrix[b].astype(f).T
        # biasT[p, h, kt, q] = (la[h]*A + ld[h]*D)^T chunked into k-tiles
        bias = (la[:, None, None] * At[None] + ld[:, None, None] * Dt[None])
        bias = bias.astype(h16).reshape(H, NKT, P, L).transpose(2, 0, 1, 3)
        in_maps.append({
            "qT": c(queries[b].astype(f).T).astype(h16),
            "kT": c(keys[b].astype(f).T).astype(h16),
            "vT": c(values[b].astype(f).T).astype(h16),
            "wqT": wqT, "wkT": wkT, "wvT": wvT, "woT": woT,
            "biasT": c(bias.reshape(P, H * NKT * L)),
            "identp": identp, "bcastw": bcastw,
            "mask01": c((attention_mask[b] > 0).astype(f).reshape(NKT, P).T),
        })
    return in_maps


def kernel(queries, keys, values, attention_mask, adjacency_matrix,
           distance_matrix, W_q, W_k, W_v, W_o, lambda_a, lambda_d, **kw):
    nc = _get_nc()
    in_maps = make_in_maps(queries, keys, values, attention_mask,
                           adjacency_matrix, distance_matrix,
                           W_q, W_k, W_v, W_o, lambda_a, lambda_d)
    res = run_bass_kernel_spmd(nc, in_maps, list(range(B)), **kw)
    outs = np.stack([res.results[i]["out"] for i in range(B)]).astype(np.float32)
    return outs
